# revision 55
# baseline (speedup 1.0000x reference)
"""Trainium2 Bass kernel for nn_ARSG (additive-attention style scoring with a
1-D conv over location features), data-parallel over batch across 8 NeuronCores.

Math (per batch b):
    f      = conv1d(F_matrix, a_prev[b])          # Toeplitz matmul over T
    x      = tanh(s_prev[b] @ Ww + hT[b] @ Vw + Vb + f @ Uw)
    e      = x @ ww
    out[b] = softmax(beta * e)

Key restructurings (validated vs the reference in fp64/fp32 mock):
  * Uw is folded into F on the host: G = F @ Uw, so Uf^T = G^T @ C_b^T where
    C_b^T is the (banded Toeplitz) conv coefficient matrix built from a_prev.
    This removes the separate f @ Uw matmul entirely (-25% FLOPs).
  * C_b^T tiles are materialized by DMA directly from a reversed, zero-padded
    copy of a_prev ("qrev") using an overlapping [1,128]x[1,512] access
    pattern.  Both matmul operands have their K-partitions reversed per
    128-block (G is block-reversed on the host), which keeps all AP steps
    positive while leaving the contraction sum unchanged.
  * h is transposed on the host to [b, DIM_H, T] so Vh^T accumulates into the
    same PSUM tile as Uf^T with K = DIM_H on partitions.
  * s_prev @ Ww + Vb (tiny) is computed on the host and applied as the
    per-partition bias of the tanh activation.
  * The conv pair (G, qrev) runs in fp8e4 with perf_mode=DoubleRow: each
    matmul contracts TWO 128-K-blocks at 2 rows/cycle (~2x bf16 ALU rate).
    The conv coefficients are scaled by SC=1024 on the host (softmax probs
    ~1e-3 would otherwise land in fp8 subnormals); Vw is pre-scaled by SC
    too so the shared PSUM is uniformly SC-scaled, and the tanh activation
    applies scale=1/SC (bias is added after the scale, so it stays unscaled).
    Host-simulated accuracy impact: rel err ~2.6e-3 (tolerance 2e-2).
    The Vh and e matmuls run as float32r (full fp32 data; reduced-precision
    PE mode, 1 cycle/row at N>=256 -- same rate as bf16, ~4x faster than
    fp32).

Everything below T/B/... is hardcoded for the problem sizes:
    T=1024, B=32, DIM_F=512, DIM_H=512, DIM_S=1024, DIM_W=512, 8 cores.
"""

import numpy as np

T, B, DIM_F, DIM_H, DIM_S, DIM_W = 1024, 32, 512, 512, 1024, 512
N_CORES = 8
B_LOC = B // N_CORES  # batches per core
QLEN = 2048           # padded length of the reversed conv-coefficient vector
SC = 1024.0           # fp8 conv coefficient scale (power of 2, undone in tanh)

_program_cache: dict[float, object] = {}


def _build_program(beta: float):
    import concourse.bass as bass
    import concourse.mybir as mybir
    import concourse.tile as tile
    from concourse import bacc

    f32 = mybir.dt.float32
    f32r = mybir.dt.float32r
    bf16 = mybir.dt.bfloat16
    fp8 = mybir.dt.float8e4
    DR = mybir.MatmulPerfMode.DoubleRow
    AFT = mybir.ActivationFunctionType

    nc = bacc.Bacc("TRN2", target_bir_lowering=False, debug=False)

    g_d = nc.dram_tensor("g", [T, DIM_W], fp8, kind="ExternalInput")
    vw_d = nc.dram_tensor("vw", [DIM_H, DIM_W], f32r, kind="ExternalInput")
    qr_d = nc.dram_tensor("qrev", [B_LOC, QLEN], fp8, kind="ExternalInput")
    # h ships as bf16 (halves the dominant DMA stream) and is upcast to f32
    # on the idle Vector engine; the Vh matmuls still run f32r so the PE
    # stream stays mixed-precision (all-low-precision streams trigger the P0
    # power downclock).
    ht_d = nc.dram_tensor("ht", [B_LOC, DIM_H, T], bf16, kind="ExternalInput")
    bias_d = nc.dram_tensor("bias", [128, B_LOC * 4], f32, kind="ExternalInput")
    # col 4 is all-ones: the lhsT of the partition-sum matmul in the e path.
    wwr_d = nc.dram_tensor("wwr", [128, 5], f32r, kind="ExternalInput")
    out_d = nc.dram_tensor("out", [B_LOC, T], f32, kind="ExternalOutput")

    NKJ = T // 128       # 8 K-blocks for the conv contraction (over j)
    NKD = DIM_H // 128   # 4 K-blocks for the Vh contraction (over d)
    NWT = DIM_W // 128   # 4 output w-tiles
    NTC = T // 512       # 2 t-chunks of 512 (PSUM bank / fp32 moving-max)

    with tile.TileContext(nc) as tc:
        with (
            tc.tile_pool(name="const", bufs=1) as const_pool,
            tc.tile_pool(name="htp", bufs=2) as ht_pool,
            tc.tile_pool(name="htbp", bufs=2) as htb_pool,
            tc.tile_pool(name="convp", bufs=3) as conv_pool,
            tc.tile_pool(name="xp", bufs=4) as x_pool,
            tc.tile_pool(name="ep", bufs=2) as e_pool,
            tc.tile_pool(name="sp", bufs=3) as s_pool,
            tc.tile_pool(name="smallp", bufs=4) as small_pool,
            tc.tile_pool(name="psx", bufs=6, space="PSUM") as psx_pool,
            tc.tile_pool(name="pse", bufs=1, space="PSUM") as pse_pool,
        ):
            # Startup-critical loads first, at per-kj granularity so the
            # first matmuls (kj=4..7 of batch 0) can begin as soon as their
            # slices land.  All conv coefficient tiles for a batch are
            # overlapping windows of qrev and live inside ONE [128, 1920]
            # window: W[p, c] = qrev[lb, p + c]; the rhs for (kj, tch) is the
            # slice W[:, 512*tch + 128*(NKJ-1-kj) :+ 512].
            def load_w(lb, split=False, eng=None):
                w_sb = conv_pool.tile([128, 1920], fp8, tag="conv",
                                      name=f"w_{lb}")
                # For batch 0 the first matmuls only need columns < 1024, so
                # splitting the load lets them start one DMA earlier.
                pieces = [(0, 1024), (1024, 1920)] if split else [(0, 1920)]
                for c0, c1 in pieces:
                    (eng or nc.sync).dma_start(
                        out=w_sb[:, c0:c1],
                        in_=bass.AP(tensor=qr_d, offset=lb * QLEN + c0,
                                    ap=[[1, 128], [1, c1 - c0]]),
                    )
                return w_sb

            # h (and vw) ship bf16 and are upcast to f32r on the Vector
            # engine (GpSimd casts measured 5x slower).  DMA descriptors stay
            # per-kd: one descriptor runs on ONE hw queue (~33GB/s), so
            # splitting is what buys DMA parallelism.  Issue from the GpSimd
            # sequencer to keep the Sync sequencer's issue slots (~600ns
            # each) for the conv-critical loads.
            def load_ht_kd(htb_sb, ht_sb, lb, kd):
                nc.sync.dma_start(
                    out=htb_sb[:, kd, :],
                    in_=bass.AP(tensor=ht_d,
                                offset=lb * DIM_H * T + kd * 128 * T,
                                ap=[[T, 128], [1, T]]),
                )
                nc.vector.tensor_copy(out=ht_sb[:, kd, :],
                                      in_=htb_sb[:, kd, :])

            def load_ht(lb, kds=None):
                htb_sb = htb_pool.tile([128, NKD, T], bf16, tag="htb",
                                       name=f"htb_{lb}")
                ht_sb = ht_pool.tile([128, NKD, T], f32r, tag="ht",
                                     name=f"ht_{lb}")
                for kd in (kds if kds is not None else range(NKD)):
                    load_ht_kd(htb_sb, ht_sb, lb, kd)
                return htb_sb, ht_sb

            g_sb = const_pool.tile([128, NKJ, DIM_W], fp8)

            def load_g(kjs):
                for kj in kjs:
                    nc.sync.dma_start(
                        out=g_sb[:, kj, :],
                        in_=bass.AP(tensor=g_d, offset=kj * 128 * DIM_W,
                                    ap=[[DIM_W, 128], [1, DIM_W]]),
                    )

            vw_sb = const_pool.tile([128, NKD, DIM_W], f32r)

            def load_vw(kds):
                for kd in kds:
                    nc.sync.dma_start(
                        out=vw_sb[:, kd, :],
                        in_=bass.AP(tensor=vw_d, offset=kd * 128 * DIM_W,
                                    ap=[[DIM_W, 128], [1, DIM_W]]),
                    )

            # HAM warmup: the PE idles ~5us here waiting for the first loads,
            # and whatever runs in the first ~3.4us of PE activity runs at the
            # cold 1.2GHz clock.  Spend that window on dummy matmuls over a
            # zeroed scratch tile so the real matmuls start at 2.4GHz.
            warm_in = const_pool.tile([128, 640], bf16)
            nc.vector.memset(warm_in[:], 0.0)
            # Startup DMA issue is spread across sequencers (descriptor issue
            # serializes at ~600ns each within one sequencer): conv-critical
            # loads (w0, g) go first on vector/sync, ht0 + casts on gpsimd.
            w0_sb = load_w(0, split=True)
            warm_ps = psx_pool.tile([128, 512], f32, tag="psx", name="warm_ps")
            # 8 cold dummies ~= the 3.4us HAM window; more would just delay
            # the first real matmul behind them.
            for _ in range(8):
                nc.tensor.matmul(warm_ps[:], warm_in[:, 0:128],
                                 warm_in[:, 128:640], start=True, stop=True)
            # Dummy activations so the Tanh/Exp act-table loads happen during
            # the startup DMA wait instead of right before the first use.
            aw_sb = small_pool.tile([1, 2], f32, tag="aw", name="aw")
            nc.scalar.activation(aw_sb[:, 0:1], warm_in[0:1, 0:1], AFT.Tanh)
            nc.scalar.activation(aw_sb[:, 1:2], warm_in[0:1, 0:1], AFT.Exp)

            load_g([6, 7])
            load_g([4, 5])
            htb0_sb, ht0_sb = load_ht(0, kds=[0])
            load_g([2, 3, 0, 1])
            load_vw([0])
            for kd in [1, 2, 3]:
                load_ht_kd(htb0_sb, ht0_sb, 0, kd)
            load_vw([1, 2, 3])
            bias_sb = const_pool.tile([128, B_LOC * 4], f32)
            nc.sync.dma_start(out=bias_sb[:], in_=bias_d.ap()[:])
            wwr_sb = const_pool.tile([128, 5], f32r)
            nc.sync.dma_start(out=wwr_sb[:], in_=wwr_d.ap()[:])

            # --- per-batch emission helpers -------------------------------
            # Conv runs in fp8e4 DoubleRow: each matmul contracts a PAIR of
            # 128-K-blocks at 2 rows/cycle.  g_sb slot order is
            # host-arranged [kj1,kj0,kj3,kj2,...] so pair p's dim1 is
            # (kj_hi=2p+1, kj_lo=2p) matching the rhs windows at
            # (c0, c0+128).  Pairs p>=2 first: at tch==0 their t=0
            # coefficients are naturally zero, so the start=True matmul
            # covers the full 512 columns; pairs p<2 (kj<4) carry junk t=0
            # coefficients (the t=0 conv output row is zero by construction
            # -- even-T padding in the reference), so they skip column 0
            # (N=511, odd rhs offset) and just accumulate.
            def emit_conv(w_sb, ps, tch, wt):
                for mi, p in enumerate([3, 2, 1, 0]):
                    c0 = 512 * tch + 128 * (NKJ - 2 - 2 * p)
                    skip = 1 if (tch == 0 and p < 2) else 0
                    rhs = bass.AP(
                        tensor=w_sb.tensor,
                        offset=w_sb.offset + c0 + skip,
                        ap=[list(w_sb.ap[0]), [128, 2], [1, 512 - skip]],
                    )
                    nc.tensor.matmul(
                        ps[:, skip:],
                        g_sb[:, 2 * p:2 * p + 2, wt * 128:(wt + 1) * 128],
                        rhs, start=(mi == 0), stop=False, perf_mode=DR,
                    )

            def emit_vh(ht_sb, ps, tch, wt, kds=None):
                for kd in (kds if kds is not None else range(NKD)):
                    nc.tensor.matmul(
                        ps[:],
                        vw_sb[:, kd, wt * 128:(wt + 1) * 128],
                        ht_sb[:, kd, tch * 512:(tch + 1) * 512],
                        start=False, stop=(kd == NKD - 1),
                    )

            def emit_act(lb, ps, x_sb, wt):
                nc.scalar.activation(
                    x_sb[:, wt, :], ps[:], AFT.Tanh,
                    bias=bias_sb[:, lb * 4 + wt: lb * 4 + wt + 1],
                    scale=1.0 / SC,
                )

            def emit_e(lb, tch, pe, x_sb, offload):
                pe_sl = pe[:, tch * 512:(tch + 1) * 512]
                if offload:
                    # Reduce over wt on the idle Vector engine (s = sum_wt
                    # x_wt * ww_wt), then one K=128 ones-matmul does the
                    # partition sum: frees ~1.4us/batch of PE time.
                    s_sb = s_pool.tile([128, 512], f32r, tag="s",
                                       name=f"s_{lb}_{tch}")
                    nc.vector.tensor_scalar_mul(
                        s_sb[:], x_sb[:, 0, :].bitcast(f32),
                        wwr_sb[:, 0:1].bitcast(f32))
                    for wt in range(1, NWT):
                        nc.vector.scalar_tensor_tensor(
                            s_sb[:], x_sb[:, wt, :].bitcast(f32),
                            wwr_sb[:, wt:wt + 1].bitcast(f32),
                            s_sb[:].bitcast(f32),
                            op0=mybir.AluOpType.mult,
                            op1=mybir.AluOpType.add)
                    nc.tensor.matmul(pe_sl, wwr_sb[:, 4:5], s_sb[:],
                                     start=True, stop=True)
                else:
                    for wt in range(NWT):
                        nc.tensor.matmul(
                            pe_sl, wwr_sb[:, wt:wt + 1], x_sb[:, wt, :],
                            start=(wt == 0), stop=(wt == NWT - 1),
                        )

            split_exp = abs(beta) <= 4.0

            def emit_exp(lb, tch, pe, p_sb, ssum2):
                # exp(beta*e) cannot overflow for small beta (|e| <~ 20), so
                # no max-subtraction pass is needed and each half can be
                # exponentiated independently, right after its e matmul.
                nc.scalar.activation(
                    p_sb[:, tch * 512:(tch + 1) * 512],
                    pe[:, tch * 512:(tch + 1) * 512],
                    AFT.Exp, scale=float(beta),
                    accum_out=ssum2[:, tch:tch + 1],
                )

            for lb in range(B_LOC):
                ht_sb = ht0_sb if lb == 0 else load_ht(lb)[1]
                w_sb = w0_sb if lb == 0 else load_w(lb, split=True)
                pe = pse_pool.tile([1, T], f32, tag="pse", name=f"pe_{lb}")
                p_sb = e_pool.tile([1, T], f32, tag="p", name=f"p_{lb}")
                ssum2 = small_pool.tile([1, 2], f32, tag="ssum",
                                        name=f"ssum_{lb}")
                offload = lb < B_LOC - 1
                for tch in range(NTC):
                    x_sb = x_pool.tile([128, NWT, 512], f32r, tag="x",
                                       name=f"x_{lb}_{tch}")
                    pss = []
                    for wt in range(NWT):
                        ps = psx_pool.tile([128, 512], f32, tag="psx",
                                           name=f"ps_{lb}_{tch}_{wt}")
                        pss.append(ps)
                        emit_conv(w_sb, ps, tch, wt)
                    if lb == 0:
                        # batch 0: consume ht0 kd-slices as they land so the
                        # PE keeps working while the tail of h streams in
                        for kd in range(NKD):
                            for wt in range(NWT):
                                emit_vh(ht_sb, pss[wt], tch, wt, kds=[kd])
                        for wt in range(NWT):
                            emit_act(lb, pss[wt], x_sb, wt)
                    else:
                        for wt in range(NWT):
                            emit_vh(ht_sb, pss[wt], tch, wt)
                            emit_act(lb, pss[wt], x_sb, wt)
                    # the last batch's tch1 e stays on the PE (the DVE hop
                    # would lengthen the exposed tail); its tch0 e is not
                    # tail-exposed, so it offloads like the others
                    emit_e(lb, tch, pe, x_sb, offload or tch == 0)
                    if split_exp:
                        emit_exp(lb, tch, pe, p_sb, ssum2)

                ssum = small_pool.tile([1, 1], f32, tag="ssumt",
                                       name=f"ssumt_{lb}")
                if split_exp:
                    nc.vector.tensor_add(ssum[:], ssum2[:, 0:1],
                                         ssum2[:, 1:2])
                else:
                    mx = small_pool.tile([1, 1], f32, tag="mx", name=f"mx_{lb}")
                    nc.vector.reduce_max(mx[:], pe[:], axis=mybir.AxisListType.X)
                    nbm = small_pool.tile([1, 1], f32, tag="nbm", name=f"nbm_{lb}")
                    nc.vector.tensor_scalar_mul(nbm[:], mx[:], -float(beta))
                    nc.scalar.activation(
                        p_sb[:], pe[:], AFT.Exp,
                        bias=nbm[:], scale=float(beta), accum_out=ssum[:],
                    )
                rec = small_pool.tile([1, 1], f32, tag="rec", name=f"rec_{lb}")
                nc.vector.reciprocal(rec[:], ssum[:])
                o_sb = e_pool.tile([1, T], f32, tag="o", name=f"o_{lb}")
                nc.vector.tensor_scalar_mul(o_sb[:], p_sb[:], rec[:])
                nc.sync.dma_start(out=out_d.ap()[lb:lb + 1, :], in_=o_sb[:])

    nc.compile()
    return nc


def _get_program(beta: float):
    if beta not in _program_cache:
        _program_cache[beta] = _build_program(beta)
    return _program_cache[beta]


def _prepare_in_maps(F, a_prev, s_prev, h, Ww, Vw, Vb, Uw, ww):
    """Host-side sharding + layout prep. Cheap (one small matmul + copies)."""
    import ml_dtypes
    e4 = ml_dtypes.float8_e4m3
    bf16 = ml_dtypes.bfloat16
    G = (F.astype(np.float64) @ Uw.astype(np.float64)).astype(np.float32)
    # Reverse each 128-row block of G so conv lhsT/rhs partition orders match,
    # then swap each even/odd kj block so DoubleRow pair p's lhsT dim1 order
    # is (kj_hi=2p+1, kj_lo=2p), matching rhs windows at (c0, c0+128).
    G_br = G.reshape(T // 128, 128, DIM_W)[:, ::-1, :]
    G_br = G_br[[1, 0, 3, 2, 5, 4, 7, 6]]
    G_br = np.ascontiguousarray(G_br.reshape(T, DIM_W)).astype(e4)
    Ws = (s_prev.astype(np.float64) @ Ww.astype(np.float64)).astype(np.float32)
    Ws = Ws + Vb[None, :]                                   # [B, DIM_W]
    # [128, 5]: cols 0-3 = ww blocks, col 4 = ones (e partition-sum lhsT)
    wwr = np.concatenate(
        [ww.reshape(4, 128).T, np.ones((128, 1), np.float32)], axis=1)
    wwr = np.ascontiguousarray(wwr)
    # Vh accumulates into the same PSUM as the SC-scaled fp8 conv, so Vw is
    # pre-scaled by SC (exact, power of 2); the tanh activation undoes it.
    Vw_c = np.ascontiguousarray(Vw * np.float32(SC))

    in_maps = []
    for core in range(N_CORES):
        b0 = core * B_LOC
        ppad = np.zeros((B_LOC, 2 * T - 1), np.float32)
        ppad[:, T // 2 - 1: T // 2 - 1 + T] = a_prev[b0:b0 + B_LOC]
        qrev = np.zeros((B_LOC, QLEN), np.float32)
        qrev[:, : 2 * T - 1] = ppad[:, ::-1]
        qrev = (qrev * np.float32(SC)).astype(e4)
        hT = np.ascontiguousarray(
            h[:, b0:b0 + B_LOC, :].transpose(1, 2, 0)).astype(bf16)
        bias_core = np.ascontiguousarray(
            Ws[b0:b0 + B_LOC].reshape(B_LOC, 4, 128).transpose(2, 0, 1)
            .reshape(128, B_LOC * 4)
        )
        in_maps.append({
            "g": G_br, "vw": Vw_c, "qrev": qrev,
            "ht": hT, "bias": bias_core, "wwr": wwr,
        })
    return in_maps


def kernel(**inputs: np.ndarray) -> np.ndarray:
    F = np.ascontiguousarray(np.asarray(inputs["F_matrix"], dtype=np.float32))
    a_prev = np.ascontiguousarray(np.asarray(inputs["a_prev"], dtype=np.float32))
    s_prev = np.ascontiguousarray(np.asarray(inputs["s_prev"], dtype=np.float32))
    h = np.ascontiguousarray(np.asarray(inputs["h"], dtype=np.float32))
    Ww = np.asarray(inputs["Ww"], dtype=np.float32)
    Vw = np.asarray(inputs["Vw"], dtype=np.float32)
    Vb = np.asarray(inputs["Vb"], dtype=np.float32)
    Uw = np.asarray(inputs["Uw"], dtype=np.float32)
    ww = np.asarray(inputs["ww"], dtype=np.float32)
    beta = float(np.asarray(inputs["beta"]))

    nc = _get_program(beta)
    in_maps = _prepare_in_maps(F, a_prev, s_prev, h, Ww, Vw, Vb, Uw, ww)

    from concourse.bass_utils import run_bass_kernel_spmd

    res = run_bass_kernel_spmd(nc, in_maps, core_ids=list(range(N_CORES)))
    out = np.concatenate(
        [res.results[i]["out"] for i in range(N_CORES)], axis=0
    ).astype(np.float32)
    return out

